# revision 1
# baseline (speedup 1.0000x reference)
"""Trainium2 Bass kernel for nn_BasicRNNBlock (vanilla tanh RNN).

Reference semantics (fp32):
    xp = einsum("bti,hi->tbh", x, W_ih) + b_ih + b_hh      # input projection
    h_t = tanh(xp_t + h_{t-1} @ W_hh.T),  h_0 = 0          # T sequential steps
    out[b, t, :] = h_t[b]                                  # [B, T, H]

Shapes: B=64, T=512, I=H=1024.  Sharding: data-parallel over batch across
8 NeuronCores (8 batches/core, weights replicated).  All-fp16 matmul inputs
(measured ~3e-4 rel error per step), fp32 PSUM accumulation.

Per-core device program (SPMD):
  The recurrence state is kept transposed (hT, [128, 64] = [kappa, chunk*8+b])
  so each step's 64 fp16 matmuls (W_hh 128x128 blocks stationary) accumulate
  z^T into PSUM directly in hT-major layout; an identity matmul injects the
  precomputed xp_t (start=True, first in the accumulation group); ACT tanh
  (split in two halves for cross-step pipelining) produces h_t^T which feeds
  the next step and is DMA'd out.  The input projection GEMM (xp) is
  interleaved into the recurrence: one projection matmul per step computes
  the next 64-step slice of xp while the current slice is consumed.
"""
import numpy as np

B, T, I, H = 64, 512, 1024, 1024
N_CORES = 8
BS = B // N_CORES          # 8 batches per core
NCH = H // 128             # 8 chunks of 128 along H
WIN = 64                   # recurrence steps per projection slice (512 cols)
NSLICE = T // WIN          # 8 projection slices


def _build_program(steps=T, interleave=True, split_tanh=True):
    from concourse import bacc, mybir
    import concourse.tile as tile

    f16 = mybir.dt.float16
    f32 = mybir.dt.float32

    nc = bacc.Bacc(None, target_bir_lowering=False)

    wih = nc.declare_dram_parameter("wih", [128, 8192], f16, isOutput=False)
    whh = nc.declare_dram_parameter("whh", [128, 8192], f16, isOutput=False)
    xt = nc.declare_dram_parameter("xt", [128, 8 * 4096], f16, isOutput=False)
    ident = nc.declare_dram_parameter("ident", [128, 128], f16, isOutput=False)
    bias = nc.declare_dram_parameter("bias", [128, 8], f32, isOutput=False)
    y = nc.declare_dram_parameter("y", [steps, 128, 64], f16, isOutput=True)

    n_slices_used = (steps + WIN - 1) // WIN

    with tile.TileContext(nc) as tc:
        with (
            tc.tile_pool(name="const", bufs=1) as const_pool,
            tc.tile_pool(name="xslice", bufs=2) as xslice_pool,
            tc.tile_pool(name="xp", bufs=3) as xp_pool,
            tc.tile_pool(name="hst", bufs=3) as h_pool,
            tc.tile_pool(name="pp", bufs=2, space="PSUM") as proj_psum,
            tc.tile_pool(name="rp", bufs=3, space="PSUM") as rec_psum,
        ):
            wih_sb = const_pool.tile([128, 8192], f16)
            whh_sb = const_pool.tile([128, 8192], f16)
            ident_sb = const_pool.tile([128, 128], f16)
            bias_sb = const_pool.tile([128, 8], f32)
            nc.sync.dma_start(wih_sb[:], wih[:])
            nc.sync.dma_start(whh_sb[:], whh[:])
            nc.sync.dma_start(ident_sb[:], ident[:])
            nc.sync.dma_start(bias_sb[:], bias[:])

            eng_cycle = [nc.sync, nc.gpsimd]

            def load_xt_slice(s):
                """DMA xt k-chunks for slice s into a fresh [128, 4096] tile."""
                xsl = xslice_pool.tile([128, 8 * 512], f16, name="xsl", tag="xsl")
                for k in range(8):
                    eng_cycle[k % 2].dma_start(
                        xsl[:, k * 512:(k + 1) * 512],
                        xt[:, k * 4096 + s * 512: k * 4096 + (s + 1) * 512],
                    )
                return xsl

            # xp slice tile layout: [kappa, c*512 + local_t*8 + b]
            def proj_block(xsl, xp_tile, c, k, psum_holder):
                if k == 0:
                    psum_holder[0] = proj_psum.tile([128, 512], f32, name="ppsum", tag="ppsum")
                nc.tensor.matmul(
                    psum_holder[0][:],
                    wih_sb[:, k * 1024 + c * 128: k * 1024 + (c + 1) * 128],
                    xsl[:, k * 512:(k + 1) * 512],
                    start=(k == 0), stop=(k == 7),
                )
                if k == 7:
                    nc.vector.tensor_scalar_add(
                        xp_tile[:, c * 512:(c + 1) * 512],
                        psum_holder[0][:],
                        bias_sb[:, c:c + 1],
                    )

            # ---------------- prologue: projection slice 0 ----------------
            xp_tiles = {}
            xsl_tiles = {}
            xsl_tiles[0] = load_xt_slice(0)
            if n_slices_used > 1:
                xsl_tiles[1] = load_xt_slice(1)
            xp_tiles[0] = xp_pool.tile([128, 8 * 512], f16, name="xpt", tag="xpt")
            ph = [None]
            for c in range(NCH):
                for k in range(8):
                    proj_block(xsl_tiles[0], xp_tiles[0], c, k, ph)

            if not interleave:
                for s in range(1, n_slices_used):
                    if s + 1 < n_slices_used and (s + 1) not in xsl_tiles:
                        xsl_tiles[s + 1] = load_xt_slice(s + 1)
                    xp_tiles[s] = xp_pool.tile([128, 8 * 512], f16, name="xpt", tag="xpt")
                    for c in range(NCH):
                        for k in range(8):
                            proj_block(xsl_tiles[s], xp_tiles[s], c, k, ph)

            # ---------------- recurrence ----------------
            h_cur = None
            pph = [None]
            for t in range(steps):
                s = t // WIN
                local = t - s * WIN
                xp3 = xp_tiles[s][:].rearrange("p (c n) -> p c n", c=NCH)

                if split_tanh:
                    psum_lo = rec_psum.tile([128, 4, 8], f32, name="pslo", tag="pslo")
                    psum_hi = rec_psum.tile([128, 4, 8], f32, name="pshi", tag="pshi")
                    nc.tensor.matmul(
                        psum_lo[:], ident_sb[:], xp3[:, 0:4, local * 8:(local + 1) * 8],
                        start=True, stop=(t == 0), skip_group_check=True)
                    nc.tensor.matmul(
                        psum_hi[:], ident_sb[:], xp3[:, 4:8, local * 8:(local + 1) * 8],
                        start=True, stop=(t == 0), skip_group_check=True)
                else:
                    psum = rec_psum.tile([128, 8, 8], f32)
                    nc.tensor.matmul(
                        psum[:], ident_sb[:],
                        xp3[:, :, local * 8:(local + 1) * 8],
                        start=True, stop=(t == 0),
                        skip_group_check=True,
                    )

                def wblock(c, k, last):
                    if split_tanh:
                        pt = psum_lo if c < 4 else psum_hi
                        out_ap = pt[:, c % 4, :]
                    else:
                        out_ap = psum[:, c, :]
                    nc.tensor.matmul(
                        out_ap,
                        whh_sb[:, k * 1024 + c * 128: k * 1024 + (c + 1) * 128],
                        h_cur[:, k * 8:(k + 1) * 8],
                        start=False, stop=last,
                        skip_group_check=True,
                    )

                if t > 0:
                    # low half: c 0-3; k 0-3 first (needs h half1), then k 4-7
                    for k in range(8):
                        for c in range(4):
                            wblock(c, k, (split_tanh and k == 7 and c == 3))
                h_new = h_pool.tile([128, 64], f16)
                if split_tanh:
                    nc.scalar.activation(
                        h_new[:, 0:32],
                        psum_lo[:].rearrange("p c n -> p (c n)"),
                        mybir.ActivationFunctionType.Tanh,
                    )
                # interleaved projection work for slice s+1
                if interleave:
                    sp = s + 1
                    if sp < n_slices_used:
                        if local == 0:
                            xp_tiles[sp] = xp_pool.tile([128, 8 * 512], f16, name="xpt", tag="xpt")
                        cp, kp = local // 8, local % 8
                        proj_block(xsl_tiles[sp], xp_tiles[sp], cp, kp, pph)
                        if 40 <= local < 48 and sp + 1 < n_slices_used:
                            if local == 40:
                                xsl_tiles[sp + 1] = xslice_pool.tile(
                                    [128, 8 * 512], f16, name="xsl", tag="xsl")
                            k = local - 40
                            eng_cycle[k % 2].dma_start(
                                xsl_tiles[sp + 1][:, k * 512:(k + 1) * 512],
                                xt[:, k * 4096 + (sp + 1) * 512:
                                   k * 4096 + (sp + 2) * 512],
                            )
                if t > 0:
                    for k in range(8):
                        for c in range(4, 8):
                            wblock(c, k, (c == 7 and k == 7))
                if split_tanh:
                    nc.scalar.activation(
                        h_new[:, 32:64],
                        psum_hi[:].rearrange("p c n -> p (c n)"),
                        mybir.ActivationFunctionType.Tanh,
                    )
                else:
                    nc.scalar.activation(
                        h_new[:], psum[:].rearrange("p c n -> p (c n)"),
                        mybir.ActivationFunctionType.Tanh,
                    )
                nc.sync.dma_start(y[t], h_new[:])
                h_cur = h_new

    nc.compile()
    return nc


_PROGRAM_CACHE = {}
BUILD_KW = {}


def _get_program(steps=T):
    key = (steps, tuple(sorted(BUILD_KW.items())))
    if key not in _PROGRAM_CACHE:
        _PROGRAM_CACHE[key] = _build_program(steps, **BUILD_KW)
    return _PROGRAM_CACHE[key]


def _prep_shared(W_ih, W_hh, b_ih, b_hh):
    # lhsT layout [kappa, k*1024 + j] = W[j, k*128+kappa]
    def to_lhsT(W):
        return np.ascontiguousarray(
            W.T.reshape(8, 128, 1024).transpose(1, 0, 2).reshape(128, 8192)
        )

    wih_np = to_lhsT(np.asarray(W_ih)).astype(np.float16)
    whh_np = to_lhsT(np.asarray(W_hh)).astype(np.float16)
    bias_np = np.ascontiguousarray(
        (np.asarray(b_ih) + np.asarray(b_hh)).astype(np.float32).reshape(8, 128).T
    )
    ident_np = np.eye(128, dtype=np.float16)
    return wih_np, whh_np, bias_np, ident_np


TRACE = False
LAST_RESULT = [None]


def kernel(x, W_ih, W_hh, b_ih, b_hh, _steps=T):
    from concourse.bass_utils import run_bass_kernel_spmd

    x = np.asarray(x)
    steps = _steps
    nc = _get_program(steps)
    wih_np, whh_np, bias_np, ident_np = _prep_shared(W_ih, W_hh, b_ih, b_hh)

    in_maps = []
    for core in range(N_CORES):
        xs = x[core * BS:(core + 1) * BS]          # [8, T, I]
        # xt[kappa, k*4096 + t*8 + b] = x[b, t, k*128+kappa]
        xt_np = np.ascontiguousarray(
            xs.transpose(2, 1, 0)                   # [I, T, B]
            .reshape(8, 128, T * BS)                # [k, kappa, t*8+b]
            .transpose(1, 0, 2)                     # [kappa, k, t*8+b]
            .reshape(128, 8 * 4096)
        ).astype(np.float16)
        in_maps.append({
            "wih": wih_np, "whh": whh_np, "xt": xt_np,
            "ident": ident_np, "bias": bias_np,
        })

    res = run_bass_kernel_spmd(nc, in_maps, list(range(N_CORES)), trace=TRACE)
    LAST_RESULT[0] = res

    out = np.empty((B, T, H), dtype=np.float32)
    for core in range(N_CORES):
        yv = res.results[core]["y"]                 # [steps, 128, 64] fp16
        hb = (
            yv.reshape(steps, 128, 8, 8)
            .transpose(3, 0, 2, 1)                  # [b, t, c, kappa]
            .reshape(BS, steps, H)
            .astype(np.float32)
        )
        out[core * BS:(core + 1) * BS, :steps] = hb
    return out



# revision 3
# speedup vs baseline: 1.0201x; 1.0201x over previous
"""Trainium2 Bass kernel for nn_BasicRNNBlock (vanilla tanh RNN).

Reference semantics (fp32):
    xp = einsum("bti,hi->tbh", x, W_ih) + b_ih + b_hh      # input projection
    h_t = tanh(xp_t + h_{t-1} @ W_hh.T),  h_0 = 0          # T sequential steps
    out[b, t, :] = h_t[b]                                  # [B, T, H]

Shapes: B=64, T=512, I=H=1024.  Sharding: data-parallel over batch across
8 NeuronCores (8 batches/core, weights replicated).  All-fp16 matmul inputs,
fp32 PSUM accumulation.

Per-core device program (SPMD), v2:
  Phase 1 (prologue): the full input projection runs as one dense burst of
  512 N=512 matmuls (HAM-warm => ~2x faster streaming than interleaved); the
  resulting xp for all T steps stays resident in SBUF (8 MB) in transposed
  layout [kappa, s*4096 + c*512 + local*8 + b].
  Phase 2 (recurrence): per step, 64 fp16 LDW+MM pairs (W_hh 128x128 blocks
  stationary, hT [128, 8] moving) accumulate z^T into two PSUM tiles; an
  identity matmul injects xp_t (start=True).  MM order is staged by k-group
  so matmuls that consume the previous step's tanh-hi half issue last:
    ident_lo, ident_hi, [k0..3 x c0..7], [k4..7 x c0..3] -> tanh_lo,
    [k4..7 x c4..7] -> tanh_hi.
  This keeps the weight-load port saturated (~27 ns/block) with no
  step-boundary stalls.
"""
import numpy as np

B, T, I, H = 64, 512, 1024, 1024
N_CORES = 8
BS = B // N_CORES          # 8 batches per core
NCH = H // 128             # 8 chunks of 128 along H
WIN = 64                   # steps per projection slice (512 cols)
NSLICE = T // WIN          # 8 projection slices


def _build_program(steps=T):
    from concourse import bacc, mybir
    import concourse.tile as tile

    f16 = mybir.dt.float16
    f32 = mybir.dt.float32

    nc = bacc.Bacc(None, target_bir_lowering=False)

    wih = nc.declare_dram_parameter("wih", [128, 8192], f16, isOutput=False)
    whh = nc.declare_dram_parameter("whh", [128, 8192], f16, isOutput=False)
    xt = nc.declare_dram_parameter("xt", [128, 8 * 4096], f16, isOutput=False)
    ident = nc.declare_dram_parameter("ident", [128, 128], f16, isOutput=False)
    bias = nc.declare_dram_parameter("bias", [128, 8], f32, isOutput=False)
    y = nc.declare_dram_parameter("y", [steps, 128, 64], f16, isOutput=True)

    n_slices_used = (steps + WIN - 1) // WIN

    with tile.TileContext(nc) as tc:
        with (
            tc.tile_pool(name="const", bufs=1) as const_pool,
            tc.tile_pool(name="xslice", bufs=2) as xslice_pool,
            tc.tile_pool(name="xpall", bufs=1) as xpall_pool,
            tc.tile_pool(name="hst", bufs=3) as h_pool,
            tc.tile_pool(name="pp", bufs=2, space="PSUM") as proj_psum,
            tc.tile_pool(name="rp", bufs=3, space="PSUM") as rec_psum,
        ):
            wih_sb = const_pool.tile([128, 8192], f16)
            whh_sb = const_pool.tile([128, 8192], f16)
            ident_sb = const_pool.tile([128, 128], f16)
            bias_sb = const_pool.tile([128, 8], f32)
            nc.sync.dma_start(wih_sb[:], wih[:])
            nc.sync.dma_start(whh_sb[:], whh[:])
            nc.sync.dma_start(ident_sb[:], ident[:])
            nc.sync.dma_start(bias_sb[:], bias[:])

            eng_cycle = [nc.sync, nc.gpsimd]

            def load_xt_slice(s):
                """DMA xt k-chunks for slice s into a fresh [128, 4096] tile."""
                xsl = xslice_pool.tile([128, 8 * 512], f16, name="xsl", tag="xsl")
                for k in range(8):
                    eng_cycle[k % 2].dma_start(
                        xsl[:, k * 512:(k + 1) * 512],
                        xt[:, k * 4096 + s * 512: k * 4096 + (s + 1) * 512],
                    )
                return xsl

            # ------------- phase 1: full input projection (dense burst) ------
            # xp_all layout: [kappa, s*4096 + c*512 + local_t*8 + b]
            xp_all = xpall_pool.tile([128, n_slices_used * 4096], f16)

            xsl_next = load_xt_slice(0)
            for s in range(n_slices_used):
                xsl = xsl_next
                if s + 1 < n_slices_used:
                    xsl_next = load_xt_slice(s + 1)
                for c in range(NCH):
                    psum = proj_psum.tile([128, 512], f32, name="ppsum", tag="ppsum")
                    for k in range(8):
                        nc.tensor.matmul(
                            psum[:],
                            wih_sb[:, k * 1024 + c * 128: k * 1024 + (c + 1) * 128],
                            xsl[:, k * 512:(k + 1) * 512],
                            start=(k == 0), stop=(k == 7),
                        )
                    nc.vector.tensor_scalar_add(
                        xp_all[:, s * 4096 + c * 512: s * 4096 + (c + 1) * 512],
                        psum[:],
                        bias_sb[:, c:c + 1],
                    )

            # ------------- phase 2: recurrence -------------------------------
            h_cur = None
            for t in range(steps):
                s = t // WIN
                local = t - s * WIN
                xp3 = xp_all[:, s * 4096:(s + 1) * 4096].rearrange(
                    "p (c n) -> p c n", c=NCH)

                psum_lo = rec_psum.tile([128, 4, 8], f32, name="pslo", tag="pslo")
                psum_hi = rec_psum.tile([128, 4, 8], f32, name="pshi", tag="pshi")
                nc.tensor.matmul(
                    psum_lo[:], ident_sb[:], xp3[:, 0:4, local * 8:(local + 1) * 8],
                    start=True, stop=(t == 0), skip_group_check=True)
                nc.tensor.matmul(
                    psum_hi[:], ident_sb[:], xp3[:, 4:8, local * 8:(local + 1) * 8],
                    start=True, stop=(t == 0), skip_group_check=True)

                def wblock(c, k, last):
                    pt = psum_lo if c < 4 else psum_hi
                    nc.tensor.matmul(
                        pt[:, c % 4, :],
                        whh_sb[:, k * 1024 + c * 128: k * 1024 + (c + 1) * 128],
                        h_cur[:, k * 8:(k + 1) * 8],
                        start=False, stop=last,
                        skip_group_check=True,
                    )

                h_new = h_pool.tile([128, 64], f16)
                if t > 0:
                    # needs prev tanh_lo only (h chunks 0..3)
                    for k in range(4):
                        for c in range(8):
                            wblock(c, k, False)
                    # needs prev tanh_hi (h chunks 4..7); finish psum_lo first
                    for k in range(4, 8):
                        for c in range(4):
                            wblock(c, k, (k == 7 and c == 3))
                nc.scalar.activation(
                    h_new[:, 0:32],
                    psum_lo[:].rearrange("p c n -> p (c n)"),
                    mybir.ActivationFunctionType.Tanh,
                )
                if t > 0:
                    for k in range(4, 8):
                        for c in range(4, 8):
                            wblock(c, k, (k == 7 and c == 7))
                nc.scalar.activation(
                    h_new[:, 32:64],
                    psum_hi[:].rearrange("p c n -> p (c n)"),
                    mybir.ActivationFunctionType.Tanh,
                )
                nc.sync.dma_start(y[t], h_new[:])
                h_cur = h_new

    nc.compile()
    return nc


_PROGRAM_CACHE = {}
BUILD_KW = {}


def _get_program(steps=T):
    key = (steps, tuple(sorted(BUILD_KW.items())))
    if key not in _PROGRAM_CACHE:
        _PROGRAM_CACHE[key] = _build_program(steps, **BUILD_KW)
    return _PROGRAM_CACHE[key]


def _prep_shared(W_ih, W_hh, b_ih, b_hh):
    # lhsT layout [kappa, k*1024 + j] = W[j, k*128+kappa]
    def to_lhsT(W):
        return np.ascontiguousarray(
            W.T.reshape(8, 128, 1024).transpose(1, 0, 2).reshape(128, 8192)
        )

    wih_np = to_lhsT(np.asarray(W_ih)).astype(np.float16)
    whh_np = to_lhsT(np.asarray(W_hh)).astype(np.float16)
    bias_np = np.ascontiguousarray(
        (np.asarray(b_ih) + np.asarray(b_hh)).astype(np.float32).reshape(8, 128).T
    )
    ident_np = np.eye(128, dtype=np.float16)
    return wih_np, whh_np, bias_np, ident_np


TRACE = False
LAST_RESULT = [None]


def kernel(x, W_ih, W_hh, b_ih, b_hh, _steps=T):
    from concourse.bass_utils import run_bass_kernel_spmd

    x = np.asarray(x)
    steps = _steps
    nc = _get_program(steps)
    wih_np, whh_np, bias_np, ident_np = _prep_shared(W_ih, W_hh, b_ih, b_hh)

    in_maps = []
    for core in range(N_CORES):
        xs = x[core * BS:(core + 1) * BS]          # [8, T, I]
        # xt[kappa, k*4096 + t*8 + b] = x[b, t, k*128+kappa]
        xt_np = np.ascontiguousarray(
            xs.transpose(2, 1, 0)                   # [I, T, B]
            .reshape(8, 128, T * BS)                # [k, kappa, t*8+b]
            .transpose(1, 0, 2)                     # [kappa, k, t*8+b]
            .reshape(128, 8 * 4096)
        ).astype(np.float16)
        in_maps.append({
            "wih": wih_np, "whh": whh_np, "xt": xt_np,
            "ident": ident_np, "bias": bias_np,
        })

    res = run_bass_kernel_spmd(nc, in_maps, list(range(N_CORES)), trace=TRACE)
    LAST_RESULT[0] = res

    out = np.empty((B, T, H), dtype=np.float32)
    for core in range(N_CORES):
        yv = res.results[core]["y"]                 # [steps, 128, 64] fp16
        hb = (
            yv.reshape(steps, 128, 8, 8)
            .transpose(3, 0, 2, 1)                  # [b, t, c, kappa]
            .reshape(BS, steps, H)
            .astype(np.float32)
        )
        out[core * BS:(core + 1) * BS, :steps] = hb
    return out


# revision 4
# speedup vs baseline: 1.1327x; 1.1103x over previous
"""Trainium2 Bass kernel for nn_BasicRNNBlock (vanilla tanh RNN).

Reference semantics (fp32):
    xp = einsum("bti,hi->tbh", x, W_ih) + b_ih + b_hh      # input projection
    h_t = tanh(xp_t + h_{t-1} @ W_hh.T),  h_0 = 0          # T sequential steps
    out[b, t, :] = h_t[b]                                  # [B, T, H]

Shapes: B=64, T=512, I=H=1024.  Sharding: data-parallel over batch across
8 NeuronCores (8 batches/core, weights replicated).  All-fp16 matmul inputs,
fp32 PSUM accumulation.

Per-core device program (SPMD), v2:
  Phase 1 (prologue): the full input projection runs as one dense burst of
  512 N=512 matmuls (HAM-warm => ~2x faster streaming than interleaved); the
  resulting xp for all T steps stays resident in SBUF (8 MB) in transposed
  layout [kappa, s*4096 + c*512 + local*8 + b].
  Phase 2 (recurrence): per step, 64 fp16 LDW+MM pairs (W_hh 128x128 blocks
  stationary, hT [128, 8] moving) accumulate z^T into two PSUM tiles; an
  identity matmul injects xp_t (start=True).  MM order is staged by k-group
  so matmuls that consume the previous step's tanh-hi half issue last:
    ident_lo, ident_hi, [k0..3 x c0..7], [k4..7 x c0..3] -> tanh_lo,
    [k4..7 x c4..7] -> tanh_hi.
  This keeps the weight-load port saturated (~27 ns/block) with no
  step-boundary stalls.
"""
import numpy as np

B, T, I, H = 64, 512, 1024, 1024
N_CORES = 8
BS = B // N_CORES          # 8 batches per core
NCH = H // 128             # 8 chunks of 128 along H
WIN = 64                   # steps per projection slice (512 cols)
NSLICE = T // WIN          # 8 projection slices


def _build_program(steps=T):
    from concourse import bacc, mybir
    import concourse.tile as tile

    f16 = mybir.dt.float16
    f32 = mybir.dt.float32

    nc = bacc.Bacc(None, target_bir_lowering=False)

    wih = nc.declare_dram_parameter("wih", [128, 8192], f16, isOutput=False)
    whh = nc.declare_dram_parameter("whh", [128, 8192], f16, isOutput=False)
    xt = nc.declare_dram_parameter("xt", [128, 8 * 4096], f16, isOutput=False)
    ident = nc.declare_dram_parameter("ident", [128, 128], f16, isOutput=False)
    bias = nc.declare_dram_parameter("bias", [128, 8], f32, isOutput=False)
    y = nc.declare_dram_parameter("y", [steps, 128, 64], f16, isOutput=True)

    n_slices_used = (steps + WIN - 1) // WIN

    with tile.TileContext(nc) as tc:
        with (
            tc.tile_pool(name="const", bufs=1) as const_pool,
            tc.tile_pool(name="xslice", bufs=2) as xslice_pool,
            tc.tile_pool(name="xpall", bufs=1) as xpall_pool,
            tc.tile_pool(name="hst", bufs=3) as h_pool,
            tc.tile_pool(name="pp", bufs=2, space="PSUM") as proj_psum,
            tc.tile_pool(name="rp", bufs=3, space="PSUM") as rec_psum,
        ):
            wih_sb = const_pool.tile([128, 8192], f16)
            whh_sb = const_pool.tile([128, 8192], f16)
            ident_sb = const_pool.tile([128, 128], f16)
            bias_sb = const_pool.tile([128, 8], f32)
            nc.sync.dma_start(wih_sb[:], wih[:])
            nc.sync.dma_start(whh_sb[:], whh[:])
            nc.sync.dma_start(ident_sb[:], ident[:])
            nc.sync.dma_start(bias_sb[:], bias[:])

            eng_cycle = [nc.sync, nc.gpsimd]

            def load_xt_slice(s):
                """DMA xt k-chunks for slice s into a fresh [128, 4096] tile."""
                xsl = xslice_pool.tile([128, 8 * 512], f16, name="xsl", tag="xsl")
                for k in range(8):
                    eng_cycle[k % 2].dma_start(
                        xsl[:, k * 512:(k + 1) * 512],
                        xt[:, k * 4096 + s * 512: k * 4096 + (s + 1) * 512],
                    )
                return xsl

            # ------------- phase 1: full input projection (dense burst) ------
            # xp_all layout: [kappa, s*4096 + c*512 + local_t*8 + b]
            xp_all = xpall_pool.tile([128, n_slices_used * 4096], f16)

            xsl_next = load_xt_slice(0)
            for s in range(n_slices_used):
                xsl = xsl_next
                if s + 1 < n_slices_used:
                    xsl_next = load_xt_slice(s + 1)
                for c in range(NCH):
                    psum = proj_psum.tile([128, 512], f32, name="ppsum", tag="ppsum")
                    for k in range(8):
                        nc.tensor.matmul(
                            psum[:],
                            wih_sb[:, k * 1024 + c * 128: k * 1024 + (c + 1) * 128],
                            xsl[:, k * 512:(k + 1) * 512],
                            start=(k == 0), stop=(k == 7),
                        )
                    nc.vector.tensor_scalar_add(
                        xp_all[:, s * 4096 + c * 512: s * 4096 + (c + 1) * 512],
                        psum[:],
                        bias_sb[:, c:c + 1],
                    )

            # ------------- phase 2: recurrence -------------------------------
            h_cur = None
            for t in range(steps):
                s = t // WIN
                local = t - s * WIN
                xp3 = xp_all[:, s * 4096:(s + 1) * 4096].rearrange(
                    "p (c n) -> p c n", c=NCH)

                psum_lo = rec_psum.tile([128, 4, 8], f32, name="pslo", tag="pslo")
                psum_hi = rec_psum.tile([128, 4, 8], f32, name="pshi", tag="pshi")
                nc.tensor.matmul(
                    psum_lo[:], ident_sb[:], xp3[:, 0:4, local * 8:(local + 1) * 8],
                    start=True, stop=(t == 0), skip_group_check=True)
                nc.tensor.matmul(
                    psum_hi[:], ident_sb[:], xp3[:, 4:8, local * 8:(local + 1) * 8],
                    start=True, stop=(t == 0), skip_group_check=True)

                def wblock(c, k, last):
                    pt = psum_lo if c < 4 else psum_hi
                    nc.tensor.matmul(
                        pt[:, c % 4, :],
                        whh_sb[:, k * 1024 + c * 128: k * 1024 + (c + 1) * 128],
                        h_cur[:, k * 8:(k + 1) * 8],
                        start=False, stop=last,
                        skip_group_check=True,
                    )

                h_new = h_pool.tile([128, 64], f16)
                if t > 0:
                    # needs prev tanh_lo only (h chunks 0..3)
                    for k in range(4):
                        for c in range(8):
                            wblock(c, k, False)
                    # needs prev tanh_hi (h chunks 4..7); finish psum_lo first
                    for k in range(4, 8):
                        for c in range(4):
                            wblock(c, k, (k == 7 and c == 3))
                nc.scalar.activation(
                    h_new[:, 0:32],
                    psum_lo[:].rearrange("p c n -> p (c n)"),
                    mybir.ActivationFunctionType.Tanh,
                )
                if t > 0:
                    for k in range(4, 8):
                        for c in range(4, 8):
                            wblock(c, k, (k == 7 and c == 7))
                nc.scalar.activation(
                    h_new[:, 32:64],
                    psum_hi[:].rearrange("p c n -> p (c n)"),
                    mybir.ActivationFunctionType.Tanh,
                )
                nc.sync.dma_start(y[t], h_new[:])
                h_cur = h_new

    nc.compile()
    return nc


def _build_program_raw(steps=T):
    """Raw-bass build (no TileContext): the tile framework increments a
    per-engine progress semaphore on EVERY instruction, and those semaphore
    writes serialize at ~34 ns each — slower than the ~27 ns LDW+MM issue
    rate, making the semaphore unit the bottleneck (measured: step period
    2255 ns == 66 MMs x 34.2 ns).  Raw bass places semaphores only on the
    real dependency edges (2 psum stops + 2 tanh + DMAs per step).
    """
    from concourse import bacc, mybir
    import concourse.bass as bass

    f16 = mybir.dt.float16
    f32 = mybir.dt.float32
    Tanh = mybir.ActivationFunctionType.Tanh

    nc = bacc.Bacc(None, target_bir_lowering=False)

    wih = nc.declare_dram_parameter("wih", [128, 8192], f16, isOutput=False)
    whh = nc.declare_dram_parameter("whh", [128, 8192], f16, isOutput=False)
    xt = nc.declare_dram_parameter("xt", [128, 8 * 4096], f16, isOutput=False)
    ident = nc.declare_dram_parameter("ident", [128, 128], f16, isOutput=False)
    bias = nc.declare_dram_parameter("bias", [128, 8], f32, isOutput=False)
    y = nc.declare_dram_parameter("y", [steps, 128, 64], f16, isOutput=True)

    n_slices = (steps + WIN - 1) // WIN

    wih_sb = nc.alloc_sbuf_tensor("wih_sb", [128, 8192], f16)
    whh_sb = nc.alloc_sbuf_tensor("whh_sb", [128, 8192], f16)
    ident_sb = nc.alloc_sbuf_tensor("ident_sb", [128, 128], f16)
    bias_sb = nc.alloc_sbuf_tensor("bias_sb", [128, 8], f32)
    xp_all = nc.alloc_sbuf_tensor("xp_all", [128, n_slices * 4096], f16)
    xsl = [nc.alloc_sbuf_tensor(f"xsl{i}", [128, 4096], f16) for i in range(2)]
    hbuf = [nc.alloc_sbuf_tensor(f"hbuf{i}", [128, 64], f16) for i in range(4)]

    ppsum = [nc.alloc_psum_tensor(f"ppsum{i}", [128, 512], f32) for i in range(2)]
    pslo = [nc.alloc_psum_tensor(f"pslo{i}", [128, 4, 8], f32) for i in range(3)]
    pshi = [nc.alloc_psum_tensor(f"pshi{i}", [128, 4, 8], f32) for i in range(3)]

    sem_const = nc.alloc_semaphore("sem_const")
    sem_xsl0 = nc.alloc_semaphore("sem_xsl0")
    sem_xsl1 = nc.alloc_semaphore("sem_xsl1")
    sem_proj = nc.alloc_semaphore("sem_proj")
    sem_xp = nc.alloc_semaphore("sem_xp")
    sem_pslo = nc.alloc_semaphore("sem_pslo")
    sem_pshi = nc.alloc_semaphore("sem_pshi")
    sem_hlo = nc.alloc_semaphore("sem_hlo")
    sem_hhi = nc.alloc_semaphore("sem_hhi")
    sem_y = nc.alloc_semaphore("sem_y")

    HB = 4   # hbuf depth

    with nc.Block() as block:

        @block.sync
        def _(sync):
            sync.dma_start(wih_sb[:], wih[:]).then_inc(sem_const, 16)
            sync.dma_start(whh_sb[:], whh[:]).then_inc(sem_const, 16)
            sync.dma_start(ident_sb[:], ident[:]).then_inc(sem_const, 16)
            sync.dma_start(bias_sb[:], bias[:]).then_inc(sem_const, 16)
            for s in range(n_slices):
                if s >= 2:
                    # xsl[s%2] consumed once proj of slice s-2 fully issued
                    sync.wait_ge(sem_proj, 8 * (s - 1))
                for k in (0, 2, 4, 6):
                    sync.dma_start(
                        xsl[s % 2][:, k * 512:(k + 1) * 512],
                        xt[:, k * 4096 + s * 512: k * 4096 + (s + 1) * 512],
                    ).then_inc(sem_xsl0, 16)
            for t in range(steps):
                sync.wait_ge(sem_hhi, t + 1)
                sync.dma_start(y[t], hbuf[t % HB][:]).then_inc(sem_y, 16)

        @block.gpsimd
        def _(gpsimd):
            for s in range(n_slices):
                if s >= 2:
                    gpsimd.wait_ge(sem_proj, 8 * (s - 1))
                for k in (1, 3, 5, 7):
                    gpsimd.dma_start(
                        xsl[s % 2][:, k * 512:(k + 1) * 512],
                        xt[:, k * 4096 + s * 512: k * 4096 + (s + 1) * 512],
                    ).then_inc(sem_xsl1, 16)

        @block.tensor
        def _(tensor):
            tensor.wait_ge(sem_const, 64)
            # ---- phase 1: input projection, dense warm burst ----
            for s in range(n_slices):
                tensor.wait_ge(sem_xsl0, 64 * (s + 1))
                tensor.wait_ge(sem_xsl1, 64 * (s + 1))
                for c in range(NCH):
                    idx = 8 * s + c
                    if idx >= 2:
                        tensor.wait_ge(sem_xp, idx - 1)  # ppsum ping-pong WAR
                    for k in range(8):
                        mm = tensor.matmul(
                            ppsum[idx % 2][:],
                            wih_sb[:, k * 1024 + c * 128: k * 1024 + (c + 1) * 128],
                            xsl[s % 2][:, k * 512:(k + 1) * 512],
                            start=(k == 0), stop=(k == 7),
                        )
                        if k == 7:
                            mm.then_inc(sem_proj, 1)

            # ---- phase 2: recurrence ----
            for t in range(steps):
                s = t // WIN
                local = t - s * WIN
                if local == 0:
                    tensor.wait_ge(sem_xp, 8 * (s + 1))
                xp3 = xp_all[:, s * 4096:(s + 1) * 4096].rearrange(
                    "p (c n) -> p c n", c=NCH)
                lo = pslo[t % 3]
                hi = pshi[t % 3]
                mm = tensor.matmul(
                    lo[:], ident_sb[:], xp3[:, 0:4, local * 8:(local + 1) * 8],
                    start=True, stop=(t == 0), skip_group_check=True)
                if t == 0:
                    mm.then_inc(sem_pslo, 1)
                mm = tensor.matmul(
                    hi[:], ident_sb[:], xp3[:, 4:8, local * 8:(local + 1) * 8],
                    start=True, stop=(t == 0), skip_group_check=True)
                if t == 0:
                    mm.then_inc(sem_pshi, 1)

                if t == 0:
                    continue
                h_prev = hbuf[(t - 1) % HB]

                def wblock(c, k, last, sem=None):
                    pt = lo if c < 4 else hi
                    mm = tensor.matmul(
                        pt[:, c % 4, :],
                        whh_sb[:, k * 1024 + c * 128: k * 1024 + (c + 1) * 128],
                        h_prev[:, k * 8:(k + 1) * 8],
                        start=False, stop=last,
                        skip_group_check=True,
                    )
                    if sem is not None:
                        mm.then_inc(sem, 1)

                # needs h chunks 0..3 only (prev tanh_lo)
                tensor.wait_ge(sem_hlo, t)
                for k in range(4):
                    for c in range(4):
                        wblock(c, k, False)
                for k in range(2):
                    for c in range(4, 8):
                        wblock(c, k, False)
                # needs h chunks 4..7 (prev tanh_hi)
                tensor.wait_ge(sem_hhi, t)
                for k in range(4, 8):
                    for c in range(4):
                        wblock(c, k, (k == 7 and c == 3),
                               sem_pslo if (k == 7 and c == 3) else None)
                for k in range(2, 4):
                    for c in range(4, 8):
                        wblock(c, k, False)
                for k in range(4, 8):
                    for c in range(4, 8):
                        wblock(c, k, (k == 7 and c == 7),
                               sem_pshi if (k == 7 and c == 7) else None)

        @block.scalar
        def _(scalar):
            for t in range(steps):
                scalar.wait_ge(sem_pslo, t + 1)
                if t >= HB:
                    scalar.wait_ge(sem_y, 16 * (t - HB + 1))  # hbuf WAR vs y-DMA
                scalar.activation(
                    hbuf[t % HB][:, 0:32],
                    pslo[t % 3][:].rearrange("p c n -> p (c n)"),
                    Tanh,
                ).then_inc(sem_hlo, 1)
                scalar.wait_ge(sem_pshi, t + 1)
                scalar.activation(
                    hbuf[t % HB][:, 32:64],
                    pshi[t % 3][:].rearrange("p c n -> p (c n)"),
                    Tanh,
                ).then_inc(sem_hhi, 1)

        @block.vector
        def _(vector):
            vector.wait_ge(sem_const, 64)
            for s in range(n_slices):
                for c in range(NCH):
                    idx = 8 * s + c
                    vector.wait_ge(sem_proj, idx + 1)
                    vector.tensor_scalar_add(
                        xp_all[:, s * 4096 + c * 512: s * 4096 + (c + 1) * 512],
                        ppsum[idx % 2][:],
                        bias_sb[:, c:c + 1],
                    ).then_inc(sem_xp, 1)

    nc.compile()
    return nc


_PROGRAM_CACHE = {}
BUILD_KW = {"raw": True}


def _get_program(steps=T):
    key = (steps, tuple(sorted(BUILD_KW.items())))
    if key not in _PROGRAM_CACHE:
        kw = dict(BUILD_KW)
        raw = kw.pop("raw", False)
        builder = _build_program_raw if raw else _build_program
        _PROGRAM_CACHE[key] = builder(steps, **kw)
    return _PROGRAM_CACHE[key]


def _prep_shared(W_ih, W_hh, b_ih, b_hh):
    # lhsT layout [kappa, k*1024 + j] = W[j, k*128+kappa]
    def to_lhsT(W):
        return np.ascontiguousarray(
            W.T.reshape(8, 128, 1024).transpose(1, 0, 2).reshape(128, 8192)
        )

    wih_np = to_lhsT(np.asarray(W_ih)).astype(np.float16)
    whh_np = to_lhsT(np.asarray(W_hh)).astype(np.float16)
    bias_np = np.ascontiguousarray(
        (np.asarray(b_ih) + np.asarray(b_hh)).astype(np.float32).reshape(8, 128).T
    )
    ident_np = np.eye(128, dtype=np.float16)
    return wih_np, whh_np, bias_np, ident_np


TRACE = False
LAST_RESULT = [None]


def kernel(x, W_ih, W_hh, b_ih, b_hh, _steps=T):
    from concourse.bass_utils import run_bass_kernel_spmd

    x = np.asarray(x)
    steps = _steps
    nc = _get_program(steps)
    wih_np, whh_np, bias_np, ident_np = _prep_shared(W_ih, W_hh, b_ih, b_hh)

    in_maps = []
    for core in range(N_CORES):
        xs = x[core * BS:(core + 1) * BS]          # [8, T, I]
        # xt[kappa, k*4096 + t*8 + b] = x[b, t, k*128+kappa]
        xt_np = np.ascontiguousarray(
            xs.transpose(2, 1, 0)                   # [I, T, B]
            .reshape(8, 128, T * BS)                # [k, kappa, t*8+b]
            .transpose(1, 0, 2)                     # [kappa, k, t*8+b]
            .reshape(128, 8 * 4096)
        ).astype(np.float16)
        in_maps.append({
            "wih": wih_np, "whh": whh_np, "xt": xt_np,
            "ident": ident_np, "bias": bias_np,
        })

    res = run_bass_kernel_spmd(nc, in_maps, list(range(N_CORES)), trace=TRACE)
    LAST_RESULT[0] = res

    out = np.empty((B, T, H), dtype=np.float32)
    for core in range(N_CORES):
        yv = res.results[core]["y"]                 # [steps, 128, 64] fp16
        hb = (
            yv.reshape(steps, 128, 8, 8)
            .transpose(3, 0, 2, 1)                  # [b, t, c, kappa]
            .reshape(BS, steps, H)
            .astype(np.float32)
        )
        out[core * BS:(core + 1) * BS, :steps] = hb
    return out


# revision 12
# speedup vs baseline: 1.1478x; 1.0134x over previous
"""Trainium2 Bass kernel for nn_BasicRNNBlock (vanilla tanh RNN).

Reference semantics (fp32):
    xp = einsum("bti,hi->tbh", x, W_ih) + b_ih + b_hh      # input projection
    h_t = tanh(xp_t + h_{t-1} @ W_hh.T),  h_0 = 0          # T sequential steps
    out[b, t, :] = h_t[b]                                  # [B, T, H]

Shapes: B=64, T=512, I=H=1024.  Sharding: data-parallel over batch across
8 NeuronCores (8 batches/core, weights replicated).  All-fp16 matmul inputs,
fp32 PSUM accumulation.

Per-core device program (SPMD), v2:
  Phase 1 (prologue): the full input projection runs as one dense burst of
  512 N=512 matmuls (HAM-warm => ~2x faster streaming than interleaved); the
  resulting xp for all T steps stays resident in SBUF (8 MB) in transposed
  layout [kappa, s*4096 + c*512 + local*8 + b].
  Phase 2 (recurrence): per step, 64 fp16 LDW+MM pairs (W_hh 128x128 blocks
  stationary, hT [128, 8] moving) accumulate z^T into two PSUM tiles; an
  identity matmul injects xp_t (start=True).  MM order is staged by k-group
  so matmuls that consume the previous step's tanh-hi half issue last:
    ident_lo, ident_hi, [k0..3 x c0..7], [k4..7 x c0..3] -> tanh_lo,
    [k4..7 x c4..7] -> tanh_hi.
  This keeps the weight-load port saturated (~27 ns/block) with no
  step-boundary stalls.
"""
import numpy as np

B, T, I, H = 64, 512, 1024, 1024
N_CORES = 8
BS = B // N_CORES          # 8 batches per core
NCH = H // 128             # 8 chunks of 128 along H
WIN = 64                   # steps per projection slice (512 cols)
NSLICE = T // WIN          # 8 projection slices


def _build_program(steps=T):
    from concourse import bacc, mybir
    import concourse.tile as tile

    f16 = mybir.dt.float16
    f32 = mybir.dt.float32

    nc = bacc.Bacc(None, target_bir_lowering=False)

    wih = nc.declare_dram_parameter("wih", [128, 8192], f16, isOutput=False)
    whh = nc.declare_dram_parameter("whh", [128, 8192], f16, isOutput=False)
    xt = nc.declare_dram_parameter("xt", [128, 8 * 4096], f16, isOutput=False)
    ident = nc.declare_dram_parameter("ident", [128, 128], f16, isOutput=False)
    bias = nc.declare_dram_parameter("bias", [128, 8], f32, isOutput=False)
    y = nc.declare_dram_parameter("y", [steps, 128, 64], f16, isOutput=True)

    n_slices_used = (steps + WIN - 1) // WIN

    with tile.TileContext(nc) as tc:
        with (
            tc.tile_pool(name="const", bufs=1) as const_pool,
            tc.tile_pool(name="xslice", bufs=2) as xslice_pool,
            tc.tile_pool(name="xpall", bufs=1) as xpall_pool,
            tc.tile_pool(name="hst", bufs=3) as h_pool,
            tc.tile_pool(name="pp", bufs=2, space="PSUM") as proj_psum,
            tc.tile_pool(name="rp", bufs=3, space="PSUM") as rec_psum,
        ):
            wih_sb = const_pool.tile([128, 8192], f16)
            whh_sb = const_pool.tile([128, 8192], f16)
            ident_sb = const_pool.tile([128, 128], f16)
            bias_sb = const_pool.tile([128, 8], f32)
            nc.sync.dma_start(wih_sb[:], wih[:])
            nc.sync.dma_start(whh_sb[:], whh[:])
            nc.sync.dma_start(ident_sb[:], ident[:])
            nc.sync.dma_start(bias_sb[:], bias[:])

            eng_cycle = [nc.sync, nc.gpsimd]

            def load_xt_slice(s):
                """DMA xt k-chunks for slice s into a fresh [128, 4096] tile."""
                xsl = xslice_pool.tile([128, 8 * 512], f16, name="xsl", tag="xsl")
                for k in range(8):
                    eng_cycle[k % 2].dma_start(
                        xsl[:, k * 512:(k + 1) * 512],
                        xt[:, k * 4096 + s * 512: k * 4096 + (s + 1) * 512],
                    )
                return xsl

            # ------------- phase 1: full input projection (dense burst) ------
            # xp_all layout: [kappa, s*4096 + c*512 + local_t*8 + b]
            xp_all = xpall_pool.tile([128, n_slices_used * 4096], f16)

            xsl_next = load_xt_slice(0)
            for s in range(n_slices_used):
                xsl = xsl_next
                if s + 1 < n_slices_used:
                    xsl_next = load_xt_slice(s + 1)
                for c in range(NCH):
                    psum = proj_psum.tile([128, 512], f32, name="ppsum", tag="ppsum")
                    for k in range(8):
                        nc.tensor.matmul(
                            psum[:],
                            wih_sb[:, k * 1024 + c * 128: k * 1024 + (c + 1) * 128],
                            xsl[:, k * 512:(k + 1) * 512],
                            start=(k == 0), stop=(k == 7),
                        )
                    nc.vector.tensor_scalar_add(
                        xp_all[:, s * 4096 + c * 512: s * 4096 + (c + 1) * 512],
                        psum[:],
                        bias_sb[:, c:c + 1],
                    )

            # ------------- phase 2: recurrence -------------------------------
            h_cur = None
            for t in range(steps):
                s = t // WIN
                local = t - s * WIN
                xp3 = xp_all[:, s * 4096:(s + 1) * 4096].rearrange(
                    "p (c n) -> p c n", c=NCH)

                psum_lo = rec_psum.tile([128, 4, 8], f32, name="pslo", tag="pslo")
                psum_hi = rec_psum.tile([128, 4, 8], f32, name="pshi", tag="pshi")
                nc.tensor.matmul(
                    psum_lo[:], ident_sb[:], xp3[:, 0:4, local * 8:(local + 1) * 8],
                    start=True, stop=(t == 0), skip_group_check=True)
                nc.tensor.matmul(
                    psum_hi[:], ident_sb[:], xp3[:, 4:8, local * 8:(local + 1) * 8],
                    start=True, stop=(t == 0), skip_group_check=True)

                def wblock(c, k, last):
                    pt = psum_lo if c < 4 else psum_hi
                    nc.tensor.matmul(
                        pt[:, c % 4, :],
                        whh_sb[:, k * 1024 + c * 128: k * 1024 + (c + 1) * 128],
                        h_cur[:, k * 8:(k + 1) * 8],
                        start=False, stop=last,
                        skip_group_check=True,
                    )

                h_new = h_pool.tile([128, 64], f16)
                if t > 0:
                    # needs prev tanh_lo only (h chunks 0..3)
                    for k in range(4):
                        for c in range(8):
                            wblock(c, k, False)
                    # needs prev tanh_hi (h chunks 4..7); finish psum_lo first
                    for k in range(4, 8):
                        for c in range(4):
                            wblock(c, k, (k == 7 and c == 3))
                nc.scalar.activation(
                    h_new[:, 0:32],
                    psum_lo[:].rearrange("p c n -> p (c n)"),
                    mybir.ActivationFunctionType.Tanh,
                )
                if t > 0:
                    for k in range(4, 8):
                        for c in range(4, 8):
                            wblock(c, k, (k == 7 and c == 7))
                nc.scalar.activation(
                    h_new[:, 32:64],
                    psum_hi[:].rearrange("p c n -> p (c n)"),
                    mybir.ActivationFunctionType.Tanh,
                )
                nc.sync.dma_start(y[t], h_new[:])
                h_cur = h_new

    nc.compile()
    return nc


def _build_program_raw(steps=T):
    """Raw-bass build (no TileContext): the tile framework increments a
    per-engine progress semaphore on EVERY instruction, and those semaphore
    writes serialize at ~34 ns each — slower than the ~27 ns LDW+MM issue
    rate, making the semaphore unit the bottleneck (measured: step period
    2255 ns == 66 MMs x 34.2 ns).  Raw bass places semaphores only on the
    real dependency edges (2 psum stops + 2 tanh + DMAs per step).
    """
    from concourse import bacc, mybir
    import concourse.bass as bass

    f16 = mybir.dt.float16
    f32 = mybir.dt.float32
    Tanh = mybir.ActivationFunctionType.Tanh

    nc = bacc.Bacc(None, target_bir_lowering=False)

    wih = nc.declare_dram_parameter("wih", [128, 8192], f16, isOutput=False)
    whh = nc.declare_dram_parameter("whh", [128, 8192], f16, isOutput=False)
    xt = nc.declare_dram_parameter("xt", [128, 8 * 4096], f16, isOutput=False)
    ident = nc.declare_dram_parameter("ident", [128, 128], f16, isOutput=False)
    bias = nc.declare_dram_parameter("bias", [128, 8], f32, isOutput=False)
    y = nc.declare_dram_parameter("y", [steps, 128, 64], f16, isOutput=True)

    n_slices = (steps + WIN - 1) // WIN

    wih_sb = nc.alloc_sbuf_tensor("wih_sb", [128, 8192], f16)
    whh_sb = nc.alloc_sbuf_tensor("whh_sb", [128, 8192], f16)
    ident_sb = nc.alloc_sbuf_tensor("ident_sb", [128, 128], f16)
    bias_sb = nc.alloc_sbuf_tensor("bias_sb", [128, 8], f32)
    xp_all = nc.alloc_sbuf_tensor("xp_all", [128, n_slices * 4096], f16)
    xsl = [nc.alloc_sbuf_tensor(f"xsl{i}", [128, 4096], f16) for i in range(2)]
    hbuf = [nc.alloc_sbuf_tensor(f"hbuf{i}", [128, 64], f16) for i in range(4)]

    ystage = nc.alloc_sbuf_tensor("ystage", [128, 2 * 64], f16)

    ppsum = [nc.alloc_psum_tensor(f"ppsum{i}", [128, 512], f32) for i in range(2)]
    pslo = [nc.alloc_psum_tensor(f"pslo{i}", [128, 4, 8], f32) for i in range(3)]
    pshi = [nc.alloc_psum_tensor(f"pshi{i}", [128, 4, 8], f32) for i in range(3)]

    sem_const = nc.alloc_semaphore("sem_const")
    sem_xsl0 = nc.alloc_semaphore("sem_xsl0")
    sem_xsl1 = nc.alloc_semaphore("sem_xsl1")
    sem_proj = nc.alloc_semaphore("sem_proj")
    sem_xp = nc.alloc_semaphore("sem_xp")
    sem_pslo = nc.alloc_semaphore("sem_pslo")
    sem_pshi = nc.alloc_semaphore("sem_pshi")
    sem_hlo = nc.alloc_semaphore("sem_hlo")
    sem_hhi = nc.alloc_semaphore("sem_hhi")
    sem_stg = nc.alloc_semaphore("sem_stg")
    sem_y = nc.alloc_semaphore("sem_y")
    sem_yg = nc.alloc_semaphore("sem_yg")

    HB = 4   # hbuf depth

    with nc.Block() as block:

        @block.sync
        def _(sync):
            sync.dma_start(wih_sb[:], wih[:]).then_inc(sem_const, 16)
            sync.dma_start(whh_sb[:], whh[:]).then_inc(sem_const, 16)
            sync.dma_start(ident_sb[:], ident[:]).then_inc(sem_const, 16)
            sync.dma_start(bias_sb[:], bias[:]).then_inc(sem_const, 16)
            for s in range(n_slices):
                if s >= 2:
                    # xsl[s%2] consumed once proj of slice s-2 fully issued
                    sync.wait_ge(sem_proj, 8 * (s - 1))
                for k in (0, 2, 4, 6):
                    sync.dma_start(
                        xsl[s % 2][:, k * 512:(k + 1) * 512],
                        xt[:, k * 4096 + s * 512: k * 4096 + (s + 1) * 512],
                    ).then_inc(sem_xsl0, 16)
            for t in range(steps):
                # output DMA reads the staging copy, not hbuf, so it never
                # contends with the PE's h-operand stream on SBUF reads
                slot = t % 2
                sync.wait_ge(sem_stg, t + 1)
                sync.dma_start(
                    y[t], ystage[:, slot * 64:(slot + 1) * 64]
                ).then_inc(sem_y, 16)

        @block.gpsimd
        def _(gpsimd):
            for s in range(n_slices):
                if s >= 2:
                    gpsimd.wait_ge(sem_proj, 8 * (s - 1))
                for k in (1, 3, 5, 7):
                    gpsimd.dma_start(
                        xsl[s % 2][:, k * 512:(k + 1) * 512],
                        xt[:, k * 4096 + s * 512: k * 4096 + (s + 1) * 512],
                    ).then_inc(sem_xsl1, 16)

        @block.tensor
        def _(tensor):
            tensor.wait_ge(sem_const, 64)
            # ---- phase 1: input projection, dense warm burst ----
            for s in range(n_slices):
                tensor.wait_ge(sem_xsl0, 64 * (s + 1))
                tensor.wait_ge(sem_xsl1, 64 * (s + 1))
                for c in range(NCH):
                    idx = 8 * s + c
                    if idx >= 2:
                        tensor.wait_ge(sem_xp, idx - 1)  # ppsum ping-pong WAR
                    for k in range(8):
                        mm = tensor.matmul(
                            ppsum[idx % 2][:],
                            wih_sb[:, k * 1024 + c * 128: k * 1024 + (c + 1) * 128],
                            xsl[s % 2][:, k * 512:(k + 1) * 512],
                            start=(k == 0), stop=(k == 7),
                        )
                        if k == 7:
                            mm.then_inc(sem_proj, 1)

            # ---- phase 2: recurrence ----
            for t in range(steps):
                s = t // WIN
                local = t - s * WIN
                if local == 0:
                    tensor.wait_ge(sem_xp, 8 * (s + 1))
                xp3 = xp_all[:, s * 4096:(s + 1) * 4096].rearrange(
                    "p (c n) -> p c n", c=NCH)
                lo = pslo[t % 3]
                hi = pshi[t % 3]
                mm = tensor.matmul(
                    lo[:], ident_sb[:], xp3[:, 0:4, local * 8:(local + 1) * 8],
                    start=True, stop=(t == 0), skip_group_check=True)
                if t == 0:
                    mm.then_inc(sem_pslo, 1)
                mm = tensor.matmul(
                    hi[:], ident_sb[:], xp3[:, 4:8, local * 8:(local + 1) * 8],
                    start=True, stop=(t == 0), skip_group_check=True)
                if t == 0:
                    mm.then_inc(sem_pshi, 1)

                if t == 0:
                    continue
                h_prev = hbuf[(t - 1) % HB]

                def wblock(c, k, last, sem=None):
                    pt = lo if c < 4 else hi
                    mm = tensor.matmul(
                        pt[:, c % 4, :],
                        whh_sb[:, k * 1024 + c * 128: k * 1024 + (c + 1) * 128],
                        h_prev[:, k * 8:(k + 1) * 8],
                        start=False, stop=last,
                        skip_group_check=True,
                    )
                    if sem is not None:
                        mm.then_inc(sem, 1)

                # needs h chunks 0..3 only (prev tanh_lo)
                tensor.wait_ge(sem_hlo, t)
                for k in range(4):
                    for c in range(4):
                        wblock(c, k, False)
                for k in range(2):
                    for c in range(4, 8):
                        wblock(c, k, False)
                # needs h chunks 4..7 (prev tanh_hi)
                tensor.wait_ge(sem_hhi, t)
                for k in range(4, 8):
                    for c in range(4):
                        wblock(c, k, (k == 7 and c == 3),
                               sem_pslo if (k == 7 and c == 3) else None)
                for k in range(2, 4):
                    for c in range(4, 8):
                        wblock(c, k, False)
                for k in range(4, 8):
                    for c in range(4, 8):
                        wblock(c, k, (k == 7 and c == 7),
                               sem_pshi if (k == 7 and c == 7) else None)

        @block.scalar
        def _(scalar):
            for t in range(steps):
                scalar.wait_ge(sem_pslo, t + 1)
                if t >= HB:
                    scalar.wait_ge(sem_stg, t - HB + 1)  # hbuf WAR vs stage copy
                scalar.activation(
                    hbuf[t % HB][:, 0:32],
                    pslo[t % 3][:].rearrange("p c n -> p (c n)"),
                    Tanh,
                ).then_inc(sem_hlo, 1)
                scalar.wait_ge(sem_pshi, t + 1)
                scalar.activation(
                    hbuf[t % HB][:, 32:64],
                    pshi[t % 3][:].rearrange("p c n -> p (c n)"),
                    Tanh,
                ).then_inc(sem_hhi, 1)

        @block.vector
        def _(vector):
            vector.wait_ge(sem_const, 64)
            for s in range(n_slices):
                for c in range(NCH):
                    idx = 8 * s + c
                    vector.wait_ge(sem_proj, idx + 1)
                    vector.tensor_scalar_add(
                        xp_all[:, s * 4096 + c * 512: s * 4096 + (c + 1) * 512],
                        ppsum[idx % 2][:],
                        bias_sb[:, c:c + 1],
                    ).then_inc(sem_xp, 1)
            for t in range(steps):
                slot = t % 2
                vector.wait_ge(sem_hhi, t + 1)
                if t >= 2:
                    vector.wait_ge(sem_y, 16 * (t - 1))   # stage slot WAR
                vector.tensor_copy(
                    ystage[:, slot * 64:(slot + 1) * 64], hbuf[t % HB][:]
                ).then_inc(sem_stg, 1)

    nc.compile()
    return nc


_PROGRAM_CACHE = {}
BUILD_KW = {"raw": True}


def _get_program(steps=T):
    key = (steps, tuple(sorted(BUILD_KW.items())))
    if key not in _PROGRAM_CACHE:
        kw = dict(BUILD_KW)
        raw = kw.pop("raw", False)
        builder = _build_program_raw if raw else _build_program
        _PROGRAM_CACHE[key] = builder(steps, **kw)
    return _PROGRAM_CACHE[key]


def _prep_shared(W_ih, W_hh, b_ih, b_hh):
    # lhsT layout [kappa, k*1024 + j] = W[j, k*128+kappa]
    def to_lhsT(W):
        return np.ascontiguousarray(
            W.T.reshape(8, 128, 1024).transpose(1, 0, 2).reshape(128, 8192)
        )

    wih_np = to_lhsT(np.asarray(W_ih)).astype(np.float16)
    whh_np = to_lhsT(np.asarray(W_hh)).astype(np.float16)
    bias_np = np.ascontiguousarray(
        (np.asarray(b_ih) + np.asarray(b_hh)).astype(np.float32).reshape(8, 128).T
    )
    ident_np = np.eye(128, dtype=np.float16)
    return wih_np, whh_np, bias_np, ident_np


TRACE = False
LAST_RESULT = [None]


def kernel(x, W_ih, W_hh, b_ih, b_hh, _steps=T):
    from concourse.bass_utils import run_bass_kernel_spmd

    x = np.asarray(x)
    steps = _steps
    nc = _get_program(steps)
    wih_np, whh_np, bias_np, ident_np = _prep_shared(W_ih, W_hh, b_ih, b_hh)

    in_maps = []
    for core in range(N_CORES):
        xs = x[core * BS:(core + 1) * BS]          # [8, T, I]
        # xt[kappa, k*4096 + t*8 + b] = x[b, t, k*128+kappa]
        xt_np = np.ascontiguousarray(
            xs.transpose(2, 1, 0)                   # [I, T, B]
            .reshape(8, 128, T * BS)                # [k, kappa, t*8+b]
            .transpose(1, 0, 2)                     # [kappa, k, t*8+b]
            .reshape(128, 8 * 4096)
        ).astype(np.float16)
        in_maps.append({
            "wih": wih_np, "whh": whh_np, "xt": xt_np,
            "ident": ident_np, "bias": bias_np,
        })

    res = run_bass_kernel_spmd(nc, in_maps, list(range(N_CORES)), trace=TRACE)
    LAST_RESULT[0] = res

    out = np.empty((B, T, H), dtype=np.float32)
    for core in range(N_CORES):
        yv = res.results[core]["y"]                 # [steps, 128, 64] fp16
        hb = (
            yv.reshape(steps, 128, 8, 8)
            .transpose(3, 0, 2, 1)                  # [b, t, c, kappa]
            .reshape(BS, steps, H)
            .astype(np.float32)
        )
        out[core * BS:(core + 1) * BS, :steps] = hb
    return out
